# revision 19
# baseline (speedup 1.0000x reference)
"""Trainium2 Bass kernel for NpuQuantizationLinear.

Reference semantics (bit-exact target):
    qx  = clip(round_half_even(x * act_scale + act_offset), -128, 127)  # int8
    acc = qx @ q_weight  (int8 x int8 -> int32 accumulation)
    out = (acc + bias_i32) * deq_scale                                   # f32

Implementation notes:
  * Sharding: rows of x (M) are split across the 8 cores.  Column-parallel
    (the hint) would replicate the 128 MB x load + the quantize work on
    every core; row-parallel loads x once total and keeps all per-core
    work 1/8th.  No collective needed either way.
  * The PE has no int8 mode, but every int8 value is exactly representable
    in bf16 and the PSUM accumulates in fp32, which is exact for integer
    partial sums below 2^24 (|acc| here is ~1e5-1e6).  So a bf16 matmul of
    the quantized operands reproduces the int32 GEMM bit-exactly.
  * Weights travel HBM->SBUF as int8 (half the bytes of bf16) and are
    upcast to bf16 on-chip by the scalar (ACT) engine, which is otherwise
    idle.  This halves weight DMA traffic so the phase-1 x stream is not
    starved (DMA is a serial ~360 GB/s resource per core).
  * Rounding: round-half-to-even is done with the classic magic-constant
    add/sub (t + 1.5*2^23 - 1.5*2^23) on the DVE, two chained fp32 ALU
    stages, each IEEE-rounded.
  * Output is computed transposed ([N, M] per core) so that bias/deq are
    per-partition scalars -> single fused tensor_scalar epilogue
    (acc + bias) * deq, one rounding, matching the reference exactly.
  * Schedule: the first warm-weight chunk and x tile lead the DMA queue so
    the PE starts ~5 us in (was 18 us); the first nt_warm=4 n-tiles run
    with the k-loop outer (8 matmuls per quantized k-tile, the max the 8
    PSUM banks allow) so the PE is busy while quantize streams; the last
    n-tile runs k-inner per m-block so its first epilogue/store hides
    under the second block's matmuls.  TimelineSim: 449 us, PE ~98% busy
    (hard floor: 2048 MMs x 213 ns at 2.4 GHz = 437 us; sustained all-core
    clock throttles to ~2.2 GHz + ~26 ns/MM weight-switch cost -> ~530 us
    sustained floor, measured 558 us/iter amortized).
  * Host-side work is layout-only: transpose/slice x, int8 swizzle of the
    weight, un-transpose of the output.
"""

import numpy as np

_NC = 8  # NeuronCores
_P = 128  # partitions
_FREE = 512  # matmul moving free dim / PSUM bank (fp32)
_MAGIC = 12582912.0  # 1.5 * 2**23, RNE round-to-int magic constant
_WCH = 8  # k-tiles per warm-phase weight upcast chunk

_nc_cache = {}


def _build_bass(
    MP, KT, NT, act_scale, act_offset, need_clip, nt_warm, body_reps=1, loop_trips=1
):
    """Emit the per-core Bass/Tile program.

    DRAM tensors (per core):
      xt     [KT, 128, MP]  f32   x-slice transposed, k striped over partitions
      w8     [NT, 128, KT, 128] int8  weight swizzled per n-tile
      bias_s [128, NT] f32   bias striped: [p, nt] = bias[nt*128 + p]
      deq_s  [128, NT] f32   deq striped likewise
      out    [NT, 128, MP] f32  transposed output: [nt, p, m] = y[m, nt*128+p]
    """
    from contextlib import ExitStack

    import concourse.mybir as mybir
    import concourse.tile as tile
    from concourse import bacc

    f32 = mybir.dt.float32
    i8 = mybir.dt.int8

    nc = bacc.Bacc("TRN2", target_bir_lowering=False, debug=False)

    xt_d = nc.dram_tensor("xt", [KT, _P, MP], f32, kind="ExternalInput").ap()
    w_d = nc.dram_tensor("w8", [NT, _P, KT, _P], i8, kind="ExternalInput").ap()
    bias_d = nc.dram_tensor("bias_s", [_P, NT], f32, kind="ExternalInput").ap()
    deq_d = nc.dram_tensor("deq_s", [_P, NT], f32, kind="ExternalInput").ap()
    out_d = nc.dram_tensor("out", [NT, _P, MP], f32, kind="ExternalOutput").ap()

    with tile.TileContext(nc) as tc, ExitStack() as ctx:
        const_pool = ctx.enter_context(tc.tile_pool(name="const", bufs=1))
        qx_pool = ctx.enter_context(tc.tile_pool(name="qxp", bufs=1))
        x_pool = ctx.enter_context(tc.tile_pool(name="xp", bufs=5))
        t_pool = ctx.enter_context(tc.tile_pool(name="tp", bufs=4))
        wc_pool = ctx.enter_context(tc.tile_pool(name="wcp", bufs=4))  # warm int8 chunks
        ws_pool = ctx.enter_context(tc.tile_pool(name="wsp", bufs=3))  # steady int8 stage
        ww_pool = ctx.enter_context(tc.tile_pool(name="wwp", bufs=4))  # warm bf16 weights
        w_pool = ctx.enter_context(tc.tile_pool(name="wp", bufs=4))  # steady bf16 weights
        o_pool = ctx.enter_context(tc.tile_pool(name="op", bufs=6))
        ps_pool = ctx.enter_context(tc.tile_pool(name="pp", bufs=8, space="PSUM"))

        # quantized-transposed activations, resident: [128, KT, MP] bf16
        qx = qx_pool.tile([_P, KT, MP], mybir.dt.bfloat16, name="qx")

        bias_t = const_pool.tile([_P, NT], f32, name="bias_t")
        deq_t = const_pool.tile([_P, NT], f32, name="deq_t")

        # body_reps > 1 unrolls the whole body; loop_trips > 1 wraps it in a
        # hardware For_i loop.  Both are for HW timing only (the per-call
        # dispatch overhead over axon is ~77 ms, far above a single exec).
        def emit_reps():
            for _rep in range(body_reps):
                _emit_body(
                    nc, tc, KT, NT, MP, nt_warm,
                    act_scale, act_offset, need_clip,
                    qx, bias_t, deq_t,
                    xt_d, w_d, bias_d, deq_d, out_d,
                    x_pool, t_pool, wc_pool, ws_pool, ww_pool, w_pool, o_pool, ps_pool,
                )

        if loop_trips > 1:
            with tc.For_i(0, loop_trips, 1):
                emit_reps()
        else:
            emit_reps()

    nc.compile()
    return nc


def _emit_body(
    nc, tc, KT, NT, MP, nt_warm,
    act_scale, act_offset, need_clip,
    qx, bias_t, deq_t,
    xt_d, w_d, bias_d, deq_d, out_d,
    x_pool, t_pool, wc_pool, ws_pool, ww_pool, w_pool, o_pool, ps_pool,
):
    import concourse.mybir as mybir

    f32 = mybir.dt.float32
    bf16 = mybir.dt.bfloat16
    i8 = mybir.dt.int8
    Alu = mybir.AluOpType
    MB = MP // _FREE
    WCH = min(_WCH, KT)

    def quantize(kt, split_dma=False):
        # Half-tile chunks aligned with the mb matmul blocks: the mb=0
        # matmuls only need qx[:, kt, :MP//2], so chunking halves the
        # quantize->first-matmul latency (subtile deps do the rest).
        xt_t = x_pool.tile([_P, MP], f32, name="xt_t")
        if split_dma:
            # first tile: land the first half sooner (critical path at start)
            nc.sync.dma_start(xt_t[:, : MP // 2], xt_d[kt, :, : MP // 2])
            nc.sync.dma_start(xt_t[:, MP // 2 :], xt_d[kt, :, MP // 2 :])
        else:
            nc.sync.dma_start(xt_t[:], xt_d[kt])
        t1 = t_pool.tile([_P, MP], f32, name="t1")
        t2 = t_pool.tile([_P, MP], f32, name="t2") if need_clip else None
        for h in range(2):
            hs = slice(h * (MP // 2), (h + 1) * (MP // 2))
            # t1 = x * act_scale (single rounding, matches jnp f32 multiply)
            nc.vector.tensor_scalar_mul(t1[:, hs], xt_t[:, hs], act_scale)
            if act_offset != 0.0:
                nc.vector.tensor_scalar_add(t1[:, hs], t1[:, hs], act_offset)
            if need_clip:
                nc.vector.tensor_scalar(
                    t2[:, hs], t1[:, hs], _MAGIC, _MAGIC, Alu.add, Alu.subtract
                )
                nc.vector.tensor_scalar(
                    qx[:, kt, hs], t2[:, hs], 127.0, -128.0, Alu.min, Alu.max
                )
            else:
                # round-half-even to integer; bf16 store of a <=2^7
                # integer is exact
                nc.vector.tensor_scalar(
                    qx[:, kt, hs], t1[:, hs], _MAGIC, _MAGIC, Alu.add, Alu.subtract
                )

    def warm_chunk(wb, nt, k0, k1):
        # int8 chunk DMA + ACT upcast into the persistent bf16 warm tile
        w8c = wc_pool.tile([_P, WCH, _P], i8, name="w8c")
        nc.sync.dma_start(w8c[:, : k1 - k0, :], w_d[nt, :, k0:k1, :])
        nc.scalar.copy(wb[:, k0:k1, :], w8c[:, : k1 - k0, :])

    def stage_full(nt):
        # whole-n-tile int8 DMA + single ACT upcast
        w8t = ws_pool.tile([_P, KT, _P], i8, name="w8t")
        nc.sync.dma_start(w8t[:], w_d[nt])
        wb = w_pool.tile([_P, KT, _P], bf16, name="wb")
        nc.scalar.copy(wb[:], w8t[:])
        return wb

    def epilogue(nt, mb, ps):
        ot = o_pool.tile([_P, _FREE], f32, name="ot")
        # (acc + bias) * deq, per-partition scalars, single instruction
        nc.vector.tensor_scalar(
            ot[:],
            ps[:],
            bias_t[:, nt : nt + 1],
            deq_t[:, nt : nt + 1],
            Alu.add,
            Alu.mult,
        )
        nc.sync.dma_start(out_d[nt, :, mb * _FREE : (mb + 1) * _FREE], ot[:])

    # ---- start: first warm chunk + x stream first, constants after ----
    # The first matmul needs qx[:,0,:512] and warm_w[0][:,0,:]; keep both
    # critical paths short: a small 4-kt chunk for each warm tile (fast ACT
    # upcasts) and a halved first x DMA.
    n_prime = min(4, KT)  # x tiles primed ahead of the warm k-loop
    warm_w = [ww_pool.tile([_P, KT, _P], bf16, name="wbw") for _ in range(nt_warm)]
    warm_chunk(warm_w[0], 0, 0, WCH)
    quantize(0, split_dma=True)
    for nt in range(1, nt_warm):
        warm_chunk(warm_w[nt], nt, 0, WCH)
    for kt in range(1, n_prime):
        quantize(kt)
    nc.sync.dma_start(bias_t[:], bias_d)
    nc.sync.dma_start(deq_t[:], deq_d)

    # Warm block: first nt_warm n-tiles with the k-loop OUTER, so the PE
    # has 2*nt_warm matmuls to issue per quantized k-tile as it lands --
    # keeps the PE busy while phase-1 quantize streams in.
    warm_ps = [
        [ps_pool.tile([_P, _FREE], f32, name="ps") for _ in range(MB)]
        for _ in range(nt_warm)
    ]
    # Chunk-c DMA+upcast batches are injected where the serial DMA queue has
    # slack: c1 during the cold-PE window (PE at 1.2 GHz consumes k-tiles at
    # ~3.4 us while an x tile takes 1.46 us), c2/c3 after enough per-kt slack
    # (8 MM/kt = 1.7 us vs 1.46 us x DMA) has accumulated to absorb them.
    chunk_sched = {1: 0, 2: WCH + 2, 3: 2 * WCH + 2}
    n_chunks = KT // WCH
    for kt in range(KT):
        if kt >= n_prime:
            quantize(kt)
        for c in range(1, n_chunks):
            if chunk_sched.get(c) == kt:
                for nt in range(nt_warm):
                    warm_chunk(warm_w[nt], nt, c * WCH, (c + 1) * WCH)
        for nt in range(nt_warm):
            for mb in range(MB):
                nc.tensor.matmul(
                    warm_ps[nt][mb][:],
                    warm_w[nt][:, kt, :],
                    qx[:, kt, mb * _FREE : (mb + 1) * _FREE],
                    start=(kt == 0),
                    stop=(kt == KT - 1),
                )
    for nt in range(nt_warm):
        for mb in range(MB):
            epilogue(nt, mb, warm_ps[nt][mb])

    # Steady state: kt outer / mb inner so consecutive matmuls share the
    # stationary W tile (HW-measured 215 vs 228 ns/MM).  One PSUM bank
    # per (nt, mb) accumulation group, both groups of an nt in flight.
    # The LAST n-tile runs mb-outer / kt-inner so its first epilogue and
    # store are hidden under the second m-block's matmuls.
    for nt in range(nt_warm, NT):
        wb = stage_full(nt)
        pss = [ps_pool.tile([_P, _FREE], f32, name="ps") for _ in range(MB)]
        if nt < NT - 1:
            for kt in range(KT):
                for mb in range(MB):
                    nc.tensor.matmul(
                        pss[mb][:],
                        wb[:, kt, :],
                        qx[:, kt, mb * _FREE : (mb + 1) * _FREE],
                        start=(kt == 0),
                        stop=(kt == KT - 1),
                    )
            for mb in range(MB):
                epilogue(nt, mb, pss[mb])
        else:
            for mb in range(MB):
                for kt in range(KT):
                    nc.tensor.matmul(
                        pss[mb][:],
                        wb[:, kt, :],
                        qx[:, kt, mb * _FREE : (mb + 1) * _FREE],
                        start=(kt == 0),
                        stop=(kt == KT - 1),
                    )
                epilogue(nt, mb, pss[mb])


def run(inputs, trace=False):
    """Full-input entry: shard, run on 8 cores, gather.  Returns (out, results)."""
    from concourse import bass_utils

    x = np.ascontiguousarray(np.asarray(inputs["x"], dtype=np.float32))
    qw = np.asarray(inputs["q_weight"])
    act_scale = float(np.asarray(inputs["act_scale"]))
    act_offset = float(np.asarray(inputs["act_offset"]))
    deq = np.asarray(inputs["deq_scale"], dtype=np.float32)
    bias = np.asarray(inputs["bias_i32"])

    M, K = x.shape
    K2, N = qw.shape
    assert K == K2 and M % _NC == 0
    MP = M // _NC
    assert MP % _FREE == 0 and K % _P == 0 and N % _P == 0
    KT, NT = K // _P, N // _P
    nt_warm = min(4, NT)

    # clip is a no-op unless |x*s + o| can reach 127.5; check the actual data
    t_max = float(np.abs(x).max()) * abs(act_scale) + abs(act_offset)
    need_clip = t_max >= 127.0
    assert np.abs(bias).max() < 2**24  # int32 bias must be f32-exact

    key = (MP, KT, NT, act_scale, act_offset, need_clip, nt_warm)
    if key not in _nc_cache:
        _nc_cache[key] = _build_bass(*key)
    nc = _nc_cache[key]

    in_maps = stage_inputs(x, qw, deq, bias, MP, KT, NT)

    for attempt in range(2):
        results = bass_utils.run_bass_kernel_spmd(
            nc, in_maps, core_ids=list(range(_NC)), trace=trace
        )
        out = np.empty((M, N), dtype=np.float32)
        for c in range(_NC):
            out[c * MP : (c + 1) * MP, :] = results.results[c]["out"].reshape(N, MP).T
        if _spot_check(out, x, qw, act_scale, act_offset, deq, bias):
            break
        # transient device/transport corruption: rerun once
    return out, results


def _spot_check(out, x, qw, act_scale, act_offset, deq, bias, n=256, seed=7):
    """Recompute a random sample of outputs exactly on the host; the kernel
    is bit-exact, so any mismatch indicates a transient execution fault."""
    M, N = out.shape
    rng = np.random.RandomState(seed)
    ii = rng.randint(0, M, n)
    jj = rng.randint(0, N, n)
    s = np.float32(act_scale)
    o = np.float32(act_offset)
    for i, j in zip(ii, jj):
        t = x[i].astype(np.float32) * s + o  # f32, same roundings as device
        q = np.clip(np.rint(t), -128.0, 127.0).astype(np.int64)
        acc = np.int64(q @ qw[:, j].astype(np.int64) + int(bias[j]))
        exp = np.float32(acc.astype(np.int32)) * np.float32(deq[j])
        if out[i, j] != exp:
            return False
    return True


def stage_inputs(x, qw, deq, bias, MP, KT, NT):
    """Host staging (layout-only transforms)."""
    w8_r = np.ascontiguousarray(
        qw.astype(np.int8).reshape(KT, _P, NT, _P).transpose(2, 1, 0, 3)
    )
    bias_r = np.ascontiguousarray(bias.astype(np.float32).reshape(NT, _P).T)
    deq_r = np.ascontiguousarray(deq.reshape(NT, _P).T)

    in_maps = []
    for c in range(_NC):
        xc = np.ascontiguousarray(x[c * MP : (c + 1) * MP, :].T.reshape(KT, _P, MP))
        in_maps.append({"xt": xc, "w8": w8_r, "bias_s": bias_r, "deq_s": deq_r})
    return in_maps


def kernel(**inputs) -> np.ndarray:
    out, _ = run(inputs, trace=False)
    return out
